# revision 12
# baseline (speedup 1.0000x reference)
"""ContextualRoIAlign Trainium2 kernel — fused group-kernel formulation.

Problem (hardcoded): B=2, C=256, H=W=56, N=64 boxes, M=8 gt boxes, P=7.
out[b,n,c,p,q] = roi_align(fm[b], box_n)[c,p,q]
                 + mean_m roi_align(fm[b], union(box_n, gt_m))[c,p,q]

roi_align separates per axis into interpolation matrices Ay, Ax
([7,dim], host-precomputed exactly like the reference), so each roi is
out_r = Ay_r @ fm @ Ax_r^T.  The whole 9-roi group sum (box + mean of
its 8 ctx unions, 1/M folded into Ax) collapses into ONE dense spatial
kernel per group:

    G_g[(h,w),(p,q)] = sum_j Ay_j[p,h] * Ax_j[q,w]          (host, ~44 MFLOP/core)
    out_g[c,(p,q)]   = sum_hw fm[c,(h,w)] * G_g[(h,w),(p,q)] (device)

The device then does a single [256 x 3136] @ [3136 x 784] matmul per
core at full 128x128 PE utilization: hw is chunked into 25 K-tiles of
128 accumulated in PSUM; fm chunk is the stationary operand (shared by
all 16 groups); G streams as the moving operand (784 = 16 groups x 49
output pixels, split in two 392-column halves to fit a PSUM bank).
~100 matmuls total instead of the ~2800 tiny per-roi matmuls of the
two-stage formulation.

Sharding: 8 cores; core k handles image k//4, box groups [16*(k%4), +16).

DMA: G ([3136, 784]) is the dominant stream; it is shipped in 5 large
super-chunks (5 K-tiles each) to stay near peak HBM bandwidth while
overlapping with PE compute. Default io dtype bf16 (G/fm magnitudes are
O(1); psum accumulates fp32); ROI_DTYPE=float32r keeps full precision
at 2x the DMA cost.
"""
import os
import numpy as np

P = 7
B, C, H, W, N, M = 2, 256, 56, 56, 64, 8
NCORES = 8
GROUPS_PER_CORE = 16
ROIS_PER_GROUP = 9          # 1 box + 8 ctx unions
R_CORE = GROUPS_PER_CORE * ROIS_PER_GROUP   # 144
HW = H * W                  # 3136
KC = 128                    # contraction tile (partition dim)
PQ = P * P                  # 49
COLS = GROUPS_PER_CORE * PQ             # 784 moving columns
COLH = COLS // 2                        # 392 (<=512 psum bank)


def _supers_for(nch):
    """DMA super-chunk schedule: small first chunk so the PE starts
    early, large steady-state chunks for DMA efficiency."""
    lst = [1, min(4, nch - 1)] if nch > 1 else [1]
    rem = nch - sum(lst)
    while rem > 0:
        s = min(5, rem)
        lst.append(s)
        rem -= s
    return tuple(lst)


# ---------------------------------------------------------------- host prep

def _axis_weights(start, length, dim):
    """Exact numpy port of the reference's _axis_weights (float32)."""
    start = start.astype(np.float32)
    length = length.astype(np.float32)
    R = start.shape[0]
    S = int(np.ceil(dim / P))
    bin_sz = length / np.float32(P)
    grid = np.ceil(length / np.float32(P)).astype(np.int32)
    g = grid.astype(np.float32)[:, None, None]
    s = np.arange(S, dtype=np.float32)
    ph = np.arange(P, dtype=np.float32)
    coord = (start[:, None, None] + ph[None, :, None] * bin_sz[:, None, None]
             + (s[None, None, :] + np.float32(0.5)) * bin_sz[:, None, None] / g)
    valid = (coord >= -1.0) & (coord <= dim)
    c = np.maximum(coord, np.float32(0.0))
    low = np.floor(c).astype(np.int32)
    hi_clamp = low >= dim - 1
    low = np.where(hi_clamp, dim - 1, low)
    high = np.where(hi_clamp, dim - 1, low + 1)
    cv = np.where(hi_clamp, low.astype(np.float32), c)
    l = cv - low.astype(np.float32)
    smask = (s[None, None, :] < g) & valid
    w = smask.astype(np.float32) / g
    w_low = ((np.float32(1.0) - l) * w).astype(np.float32)
    w_high = (l * w).astype(np.float32)
    A = np.zeros((R, P, dim), dtype=np.float32)
    r_idx = np.broadcast_to(np.arange(R)[:, None, None], low.shape)
    p_idx = np.broadcast_to(np.arange(P)[None, :, None], low.shape)
    np.add.at(A, (r_idx, p_idx, low), w_low)
    np.add.at(A, (r_idx, p_idx, high), w_high)
    return A


def _prep_core(fm_b, boxes_b, gt_b, g0):
    """Per-core raw arrays: fmhw [3136, 256], Ghw [3136, 784] (fp32)."""
    b = boxes_b.astype(np.float32)
    g = gt_b.astype(np.float32)
    x1 = np.minimum(b[:, None, 0], g[None, :, 0])
    y1 = np.minimum(b[:, None, 1], g[None, :, 1])
    x2 = np.maximum(b[:, None, 2], g[None, :, 2])
    y2 = np.maximum(b[:, None, 3], g[None, :, 3])
    ctx = np.stack([x1, y1, x2, y2], axis=-1)                 # [N,M,4]
    rois = np.concatenate([b[:, None, :], ctx], axis=1)       # [N,9,4]
    wts = np.full((N, ROIS_PER_GROUP), np.float32(1.0 / M), dtype=np.float32)
    wts[:, 0] = np.float32(1.0)

    rois = rois[g0:g0 + GROUPS_PER_CORE].reshape(R_CORE, 4)
    wts = wts[g0:g0 + GROUPS_PER_CORE].reshape(R_CORE)
    x1, y1, x2, y2 = rois[:, 0], rois[:, 1], rois[:, 2], rois[:, 3]
    roi_w = np.maximum(x2 - x1, np.float32(1.0))
    roi_h = np.maximum(y2 - y1, np.float32(1.0))
    Ay = _axis_weights(y1, roi_h, H)                          # [144,7,56]
    Ax = _axis_weights(x1, roi_w, W) * wts[:, None, None]     # [144,7,56]

    # G_g[p,h,q,w] = sum_j Ay[j,p,h] Ax[j,q,w]  (rank-9 per group)
    Ayg = Ay.reshape(GROUPS_PER_CORE, ROIS_PER_GROUP, P * H)
    Axg = Ax.reshape(GROUPS_PER_CORE, ROIS_PER_GROUP, P * W)
    G2 = np.matmul(Ayg.transpose(0, 2, 1), Axg)               # [16, 392, 392]
    G5 = G2.reshape(GROUPS_PER_CORE, P, H, P, W)
    # -> [(h,w), (g,p,q)]
    Ghw = np.ascontiguousarray(G5.transpose(2, 4, 0, 1, 3)).reshape(HW, COLS)
    fmhw = np.ascontiguousarray(fm_b.reshape(C, HW).T)        # [(h,w), c]
    return fmhw, Ghw


def _pack_core(fmhw, Ghw, rows, nch, np_dt):
    """Gather nonzero rows, pad to nch*128, tile to [128, nch, *]."""
    pad = nch * KC
    Gp = np.zeros((pad, COLS), dtype=np.float32)
    Gp[:rows.shape[0]] = Ghw[rows]
    Fp = np.zeros((pad, C), dtype=np.float32)
    Fp[:rows.shape[0]] = fmhw[rows]
    G = np.ascontiguousarray(
        Gp.reshape(nch, KC, COLS).transpose(1, 0, 2)).astype(np_dt)
    FM = np.ascontiguousarray(
        Fp.reshape(nch, KC, C).transpose(1, 0, 2)).astype(np_dt)
    return FM, G


def _unpack_core_out(OUT):
    """OUT [128, 2, 2, 392] -> [16, 256, 7, 7]."""
    a = np.asarray(OUT, dtype=np.float32)
    a = a.reshape(128, 2, 2, COLH).transpose(1, 0, 2, 3).reshape(C, COLS)
    a = a.reshape(C, GROUPS_PER_CORE, P, P).transpose(1, 0, 2, 3)
    return np.ascontiguousarray(a)


# ---------------------------------------------------------------- program

_PROGRAMS = {}


def _build_program(dt_name, nch):
    import concourse.bacc as bacc
    import concourse.tile as tile
    import concourse.mybir as mybir

    f32 = mybir.dt.float32
    dts = {"float32": mybir.dt.float32, "float32r": mybir.dt.float32r,
           "bfloat16": mybir.dt.bfloat16}
    io_dt = dts[dt_name]
    supers = _supers_for(nch)

    nc = bacc.Bacc("TRN2", target_bir_lowering=False, debug=False,
                   enable_asserts=False)
    fm_d = nc.dram_tensor("fm", [KC, nch, C], io_dt, kind="ExternalInput").ap()
    g_d = nc.dram_tensor("g", [KC, nch, COLS], io_dt, kind="ExternalInput").ap()
    out_d = nc.dram_tensor("out", [128, 2, 2, COLH], io_dt,
                           kind="ExternalOutput").ap()

    with tile.TileContext(nc) as tc:
        with tc.tile_pool(name="fmp", bufs=1) as fmp, \
             tc.tile_pool(name="gp", bufs=1) as gpool, \
             tc.tile_pool(name="outp", bufs=1) as opool, \
             tc.tile_pool(name="psp", bufs=1, space="PSUM") as psp:

            # stream inputs in super-chunks, alternating the two HWDGE
            # rings (sync=SP, scalar=Act) so one ring's transfer hides the
            # other's ~2us completion latency; G_i and fm_i ride opposite
            # rings so they land together. The first (1-chunk) G is split
            # across both rings to minimize PE start latency.
            fmt = []
            gt = []
            c0 = 0
            for i, s in enumerate(supers):
                qa = nc.sync if i % 2 == 0 else nc.scalar
                qb = nc.scalar if i % 2 == 0 else nc.sync
                Fs = fmp.tile([KC, s, C], io_dt, name=f"fs{i}")
                qa.dma_start(Fs[:], fm_d[:, c0:c0 + s, :])
                Gs = gpool.tile([KC, s, COLS], io_dt, name=f"gs{i}")
                if i == 0:
                    nc.scalar.dma_start(Gs[:, :, 0:COLH],
                                        g_d[:, c0:c0 + s, 0:COLH])
                    nc.sync.dma_start(Gs[:, :, COLH:COLS],
                                      g_d[:, c0:c0 + s, COLH:COLS])
                else:
                    qb.dma_start(Gs[:], g_d[:, c0:c0 + s, :])
                fmt.append(Fs)
                gt.append(Gs)
                c0 += s

            ps = [psp.tile([128, COLH], f32, name=f"ps{i}") for i in range(4)]
            OUTt = [opool.tile([128, COLH], io_dt, name=f"out{i}")
                    for i in range(4)]

            def drain(i):
                # psum -> bf16 sbuf -> hbm as soon as tile i's accumulation
                # closes; overlaps the remaining matmuls
                ch_, colh_ = divmod(i, 2)
                if i % 2 == 0:
                    nc.vector.tensor_copy(out=OUTt[i][:], in_=ps[i][:])
                    nc.sync.dma_start(out_d[:, ch_, colh_, :], OUTt[i][:])
                else:
                    nc.scalar.copy(out=OUTt[i][:], in_=ps[i][:])
                    nc.scalar.dma_start(out_d[:, ch_, colh_, :], OUTt[i][:])

            chunk = 0
            last_sup = len(supers) - 1
            for sup, s in enumerate(supers):
                if sup < last_sup:
                    for j in range(s):
                        for ch in range(2):
                            lhsT = fmt[sup][:, j, ch * 128:(ch + 1) * 128]
                            for colh in range(2):
                                nc.tensor.matmul(
                                    ps[ch * 2 + colh][:],
                                    lhsT,
                                    gt[sup][:, j, colh * COLH:(colh + 1) * COLH],
                                    start=(chunk == 0), stop=False)
                        chunk += 1
                else:
                    # last super-chunk: ch-major so ps[0]/ps[1] close (and
                    # start draining) while ps[2]/ps[3] still accumulate
                    for ch in range(2):
                        for j in range(s):
                            lhsT = fmt[sup][:, j, ch * 128:(ch + 1) * 128]
                            for colh in range(2):
                                nc.tensor.matmul(
                                    ps[ch * 2 + colh][:],
                                    lhsT,
                                    gt[sup][:, j, colh * COLH:(colh + 1) * COLH],
                                    start=(chunk + j == 0), stop=(j == s - 1))
                                if j == s - 1:
                                    drain(ch * 2 + colh)
                    chunk += s

    nc.compile()
    return nc


LAST_RESULT = None


def _ensure_axon_hooks_shim():
    """concourse's axon trace path imports antenv.axon_hooks, which this
    image's antenv package lacks; provide a minimal registry so a stray
    BASS_TRACE=1 in the environment cannot crash the kernel."""
    try:
        import antenv  # noqa: F401
        import antenv.axon_hooks  # noqa: F401
        return
    except ImportError:
        pass
    try:
        import sys
        import types
        import antenv
        mod = types.ModuleType("antenv.axon_hooks")
        mod._hook = None
        mod.get_axon_ntff_profile_hook = lambda: mod._hook

        def _set(h):
            mod._hook = h

        mod.set_axon_ntff_profile_hook = _set
        sys.modules["antenv.axon_hooks"] = mod
        antenv.axon_hooks = mod
    except Exception:
        pass


def kernel(feature_map, boxes, gt_boxes):
    global LAST_RESULT
    _ensure_axon_hooks_shim()
    feature_map = np.asarray(feature_map, dtype=np.float32)
    boxes = np.asarray(boxes, dtype=np.float32)
    gt_boxes = np.asarray(gt_boxes, dtype=np.float32)

    from concourse.bass_utils import run_bass_kernel_spmd

    dt_name = os.environ.get("ROI_DTYPE", "bfloat16")
    if dt_name == "bfloat16":
        import ml_dtypes
        np_dt = ml_dtypes.bfloat16
    else:
        np_dt = np.float32

    # host prep + row compaction: drop (h,w) rows where G is all-zero
    # (outside every roi's bilinear support); all cores share one program,
    # so the chunk count is the max over cores
    raw = []
    rows_l = []
    for k in range(NCORES):
        b = k // 4
        g0 = (k % 4) * GROUPS_PER_CORE
        fmhw, Ghw = _prep_core(feature_map[b], boxes[b], gt_boxes[b], g0)
        rows = np.flatnonzero(np.any(Ghw != 0.0, axis=1))
        raw.append((fmhw, Ghw))
        rows_l.append(rows)
    nch = max(2, -(-max(r.shape[0] for r in rows_l) // KC))

    key = (dt_name, nch)
    if key not in _PROGRAMS:
        _PROGRAMS[key] = _build_program(dt_name, nch)
    nc = _PROGRAMS[key]

    in_maps = []
    for k in range(NCORES):
        FM, G = _pack_core(raw[k][0], raw[k][1], rows_l[k], nch, np_dt)
        in_maps.append({"fm": FM, "g": G})

    trace = bool(int(os.environ.get("ROI_TRACE", "0")))
    res = run_bass_kernel_spmd(nc, in_maps, list(range(NCORES)), trace=trace)
    LAST_RESULT = res

    out = np.zeros((B, N, C, P, P), dtype=np.float32)
    for k in range(NCORES):
        b = k // 4
        g0 = (k % 4) * GROUPS_PER_CORE
        out[b, g0:g0 + GROUPS_PER_CORE] = _unpack_core_out(res.results[k]["out"])
    return out


# revision 14
# speedup vs baseline: 1.0421x; 1.0421x over previous
"""ContextualRoIAlign Trainium2 kernel — fused group-kernel formulation.

Problem (hardcoded): B=2, C=256, H=W=56, N=64 boxes, M=8 gt boxes, P=7.
out[b,n,c,p,q] = roi_align(fm[b], box_n)[c,p,q]
                 + mean_m roi_align(fm[b], union(box_n, gt_m))[c,p,q]

roi_align separates per axis into interpolation matrices Ay, Ax
([7,dim], host-precomputed exactly like the reference), so each roi is
out_r = Ay_r @ fm @ Ax_r^T.  The whole 9-roi group sum (box + mean of
its 8 ctx unions, 1/M folded into Ax) collapses into ONE dense spatial
kernel per group:

    G_g[(h,w),(p,q)] = sum_j Ay_j[p,h] * Ax_j[q,w]          (host, ~44 MFLOP/core)
    out_g[c,(p,q)]   = sum_hw fm[c,(h,w)] * G_g[(h,w),(p,q)] (device)

The device then does a single [256 x 3136] @ [3136 x 784] matmul per
core at full 128x128 PE utilization: hw is chunked into 25 K-tiles of
128 accumulated in PSUM; fm chunk is the stationary operand (shared by
all 16 groups); G streams as the moving operand (784 = 16 groups x 49
output pixels, split in two 392-column halves to fit a PSUM bank).
~100 matmuls total instead of the ~2800 tiny per-roi matmuls of the
two-stage formulation.

Sharding: 8 cores; core k handles image k//4, box groups [16*(k%4), +16).

DMA: G ([3136, 784]) is the dominant stream; it is shipped in 5 large
super-chunks (5 K-tiles each) to stay near peak HBM bandwidth while
overlapping with PE compute. Default io dtype bf16 (G/fm magnitudes are
O(1); psum accumulates fp32); ROI_DTYPE=float32r keeps full precision
at 2x the DMA cost.
"""
import os
import numpy as np

P = 7
B, C, H, W, N, M = 2, 256, 56, 56, 64, 8
NCORES = 8
GROUPS_PER_CORE = 16
ROIS_PER_GROUP = 9          # 1 box + 8 ctx unions
R_CORE = GROUPS_PER_CORE * ROIS_PER_GROUP   # 144
HW = H * W                  # 3136
KC = 128                    # contraction tile (partition dim)
PQ = P * P                  # 49
COLS = GROUPS_PER_CORE * PQ             # 784 moving columns
COLH = COLS // 2                        # 392 (<=512 psum bank)


def _supers_for(nch):
    """DMA super-chunk schedule: small first chunk so the PE starts
    early, large steady-state chunks for DMA efficiency."""
    lst = [1, min(4, nch - 1)] if nch > 1 else [1]
    rem = nch - sum(lst)
    while rem > 0:
        s = min(5, rem)
        lst.append(s)
        rem -= s
    return tuple(lst)


# ---------------------------------------------------------------- host prep

def _axis_weights(start, length, dim):
    """Exact numpy port of the reference's _axis_weights (float32)."""
    start = start.astype(np.float32)
    length = length.astype(np.float32)
    R = start.shape[0]
    S = int(np.ceil(dim / P))
    bin_sz = length / np.float32(P)
    grid = np.ceil(length / np.float32(P)).astype(np.int32)
    g = grid.astype(np.float32)[:, None, None]
    s = np.arange(S, dtype=np.float32)
    ph = np.arange(P, dtype=np.float32)
    coord = (start[:, None, None] + ph[None, :, None] * bin_sz[:, None, None]
             + (s[None, None, :] + np.float32(0.5)) * bin_sz[:, None, None] / g)
    valid = (coord >= -1.0) & (coord <= dim)
    c = np.maximum(coord, np.float32(0.0))
    low = np.floor(c).astype(np.int32)
    hi_clamp = low >= dim - 1
    low = np.where(hi_clamp, dim - 1, low)
    high = np.where(hi_clamp, dim - 1, low + 1)
    cv = np.where(hi_clamp, low.astype(np.float32), c)
    l = cv - low.astype(np.float32)
    smask = (s[None, None, :] < g) & valid
    w = smask.astype(np.float32) / g
    w_low = ((np.float32(1.0) - l) * w).astype(np.float32)
    w_high = (l * w).astype(np.float32)
    A = np.zeros((R, P, dim), dtype=np.float32)
    r_idx = np.broadcast_to(np.arange(R)[:, None, None], low.shape)
    p_idx = np.broadcast_to(np.arange(P)[None, :, None], low.shape)
    np.add.at(A, (r_idx, p_idx, low), w_low)
    np.add.at(A, (r_idx, p_idx, high), w_high)
    return A


def _prep_core(fm_b, boxes_b, gt_b, g0):
    """Per-core raw arrays: fmhw [3136, 256], Ghw [3136, 784] (fp32)."""
    b = boxes_b.astype(np.float32)
    g = gt_b.astype(np.float32)
    x1 = np.minimum(b[:, None, 0], g[None, :, 0])
    y1 = np.minimum(b[:, None, 1], g[None, :, 1])
    x2 = np.maximum(b[:, None, 2], g[None, :, 2])
    y2 = np.maximum(b[:, None, 3], g[None, :, 3])
    ctx = np.stack([x1, y1, x2, y2], axis=-1)                 # [N,M,4]
    rois = np.concatenate([b[:, None, :], ctx], axis=1)       # [N,9,4]
    wts = np.full((N, ROIS_PER_GROUP), np.float32(1.0 / M), dtype=np.float32)
    wts[:, 0] = np.float32(1.0)

    rois = rois[g0:g0 + GROUPS_PER_CORE].reshape(R_CORE, 4)
    wts = wts[g0:g0 + GROUPS_PER_CORE].reshape(R_CORE)
    x1, y1, x2, y2 = rois[:, 0], rois[:, 1], rois[:, 2], rois[:, 3]
    roi_w = np.maximum(x2 - x1, np.float32(1.0))
    roi_h = np.maximum(y2 - y1, np.float32(1.0))
    Ay = _axis_weights(y1, roi_h, H)                          # [144,7,56]
    Ax = _axis_weights(x1, roi_w, W) * wts[:, None, None]     # [144,7,56]

    # G_g[p,h,q,w] = sum_j Ay[j,p,h] Ax[j,q,w]  (rank-9 per group)
    Ayg = Ay.reshape(GROUPS_PER_CORE, ROIS_PER_GROUP, P * H)
    Axg = Ax.reshape(GROUPS_PER_CORE, ROIS_PER_GROUP, P * W)
    G2 = np.matmul(Ayg.transpose(0, 2, 1), Axg)               # [16, 392, 392]
    G5 = G2.reshape(GROUPS_PER_CORE, P, H, P, W)
    # -> [(h,w), (g,p,q)]
    Ghw = np.ascontiguousarray(G5.transpose(2, 4, 0, 1, 3)).reshape(HW, COLS)
    fmhw = np.ascontiguousarray(fm_b.reshape(C, HW).T)        # [(h,w), c]
    return fmhw, Ghw


def _pack_core(fmhw, Ghw, rows, nch, np_dt):
    """Gather nonzero rows, pad to nch*128, tile to device layouts
    FM [128, nch, 256] and G [128, 2, nch, 392] (column-half major so
    each half streams as one contiguous DMA per ring)."""
    pad = nch * KC
    Gp = np.zeros((pad, COLS), dtype=np.float32)
    Gp[:rows.shape[0]] = Ghw[rows]
    Fp = np.zeros((pad, C), dtype=np.float32)
    Fp[:rows.shape[0]] = fmhw[rows]
    G = np.ascontiguousarray(
        Gp.reshape(nch, KC, 2, COLH).transpose(1, 2, 0, 3)).astype(np_dt)
    FM = np.ascontiguousarray(
        Fp.reshape(nch, KC, C).transpose(1, 0, 2)).astype(np_dt)
    return FM, G


def _unpack_core_out(OUT):
    """OUT [128, 2, 2, 392] -> [16, 256, 7, 7]."""
    a = np.asarray(OUT, dtype=np.float32)
    a = a.reshape(128, 2, 2, COLH).transpose(1, 0, 2, 3).reshape(C, COLS)
    a = a.reshape(C, GROUPS_PER_CORE, P, P).transpose(1, 0, 2, 3)
    return np.ascontiguousarray(a)


# ---------------------------------------------------------------- program

_PROGRAMS = {}


def _build_program(dt_name, nch):
    import concourse.bacc as bacc
    import concourse.tile as tile
    import concourse.mybir as mybir

    f32 = mybir.dt.float32
    dts = {"float32": mybir.dt.float32, "float32r": mybir.dt.float32r,
           "bfloat16": mybir.dt.bfloat16}
    io_dt = dts[dt_name]
    supers = _supers_for(nch)

    nc = bacc.Bacc("TRN2", target_bir_lowering=False, debug=False,
                   enable_asserts=False)
    fm_d = nc.dram_tensor("fm", [KC, nch, C], io_dt, kind="ExternalInput").ap()
    g_d = nc.dram_tensor("g", [KC, 2, nch, COLH], io_dt,
                         kind="ExternalInput").ap()
    out_d = nc.dram_tensor("out", [128, 2, 2, COLH], io_dt,
                           kind="ExternalOutput").ap()

    with tile.TileContext(nc) as tc:
        with tc.tile_pool(name="fmp", bufs=1) as fmp, \
             tc.tile_pool(name="gp", bufs=1) as gpool, \
             tc.tile_pool(name="outp", bufs=1) as opool, \
             tc.tile_pool(name="psp", bufs=1, space="PSUM") as psp:

            # stream inputs in super-chunks. The SDMA engines round-robin
            # between the two HWDGE rings (sync=SP, scalar=Act) at packet
            # granularity, so each G super-chunk is split by column half
            # across BOTH rings: the aggregate bandwidth always serves the
            # next-needed chunk and delivery stays in consumption order.
            fmt = []
            gt = []
            c0 = 0
            for i, s in enumerate(supers):
                qa = nc.sync if i % 2 == 0 else nc.scalar
                Fs = fmp.tile([KC, s, C], io_dt, name=f"fs{i}")
                qa.dma_start(Fs[:], fm_d[:, c0:c0 + s, :])
                Ga = gpool.tile([KC, s, COLH], io_dt, name=f"ga{i}")
                nc.scalar.dma_start(Ga[:], g_d[:, 0, c0:c0 + s, :])
                Gb = gpool.tile([KC, s, COLH], io_dt, name=f"gb{i}")
                nc.sync.dma_start(Gb[:], g_d[:, 1, c0:c0 + s, :])
                fmt.append(Fs)
                gt.append((Ga, Gb))
                c0 += s

            ps = [psp.tile([128, COLH], f32, name=f"ps{i}") for i in range(4)]
            OUTt = [opool.tile([128, COLH], io_dt, name=f"out{i}")
                    for i in range(4)]

            def drain(i):
                # psum -> bf16 sbuf -> hbm as soon as tile i's accumulation
                # closes; overlaps the remaining matmuls
                ch_, colh_ = divmod(i, 2)
                if i % 2 == 0:
                    nc.vector.tensor_copy(out=OUTt[i][:], in_=ps[i][:])
                    nc.sync.dma_start(out_d[:, ch_, colh_, :], OUTt[i][:])
                else:
                    nc.scalar.copy(out=OUTt[i][:], in_=ps[i][:])
                    nc.scalar.dma_start(out_d[:, ch_, colh_, :], OUTt[i][:])

            chunk = 0
            last_sup = len(supers) - 1
            for sup, s in enumerate(supers):
                if sup < last_sup:
                    for j in range(s):
                        for ch in range(2):
                            lhsT = fmt[sup][:, j, ch * 128:(ch + 1) * 128]
                            for colh in range(2):
                                nc.tensor.matmul(
                                    ps[ch * 2 + colh][:],
                                    lhsT,
                                    gt[sup][colh][:, j, :],
                                    start=(chunk == 0), stop=False)
                        chunk += 1
                else:
                    # last super-chunk: ch-major so ps[0]/ps[1] close (and
                    # start draining) while ps[2]/ps[3] still accumulate
                    for ch in range(2):
                        for j in range(s):
                            lhsT = fmt[sup][:, j, ch * 128:(ch + 1) * 128]
                            for colh in range(2):
                                nc.tensor.matmul(
                                    ps[ch * 2 + colh][:],
                                    lhsT,
                                    gt[sup][colh][:, j, :],
                                    start=False, stop=(j == s - 1))
                                if j == s - 1:
                                    drain(ch * 2 + colh)
                    chunk += s

    nc.compile()
    return nc


LAST_RESULT = None


def _ensure_axon_hooks_shim():
    """concourse's axon trace path imports antenv.axon_hooks, which this
    image's antenv package lacks; provide a minimal registry so a stray
    BASS_TRACE=1 in the environment cannot crash the kernel."""
    try:
        import antenv  # noqa: F401
        import antenv.axon_hooks  # noqa: F401
        return
    except ImportError:
        pass
    try:
        import sys
        import types
        import antenv
        mod = types.ModuleType("antenv.axon_hooks")
        mod._hook = None
        mod.get_axon_ntff_profile_hook = lambda: mod._hook

        def _set(h):
            mod._hook = h

        mod.set_axon_ntff_profile_hook = _set
        sys.modules["antenv.axon_hooks"] = mod
        antenv.axon_hooks = mod
    except Exception:
        pass


def kernel(feature_map, boxes, gt_boxes):
    global LAST_RESULT
    _ensure_axon_hooks_shim()
    feature_map = np.asarray(feature_map, dtype=np.float32)
    boxes = np.asarray(boxes, dtype=np.float32)
    gt_boxes = np.asarray(gt_boxes, dtype=np.float32)

    from concourse.bass_utils import run_bass_kernel_spmd

    dt_name = os.environ.get("ROI_DTYPE", "bfloat16")
    if dt_name == "bfloat16":
        import ml_dtypes
        np_dt = ml_dtypes.bfloat16
    else:
        np_dt = np.float32

    # host prep + row compaction: drop (h,w) rows where G is all-zero
    # (outside every roi's bilinear support); all cores share one program,
    # so the chunk count is the max over cores
    raw = []
    rows_l = []
    for k in range(NCORES):
        b = k // 4
        g0 = (k % 4) * GROUPS_PER_CORE
        fmhw, Ghw = _prep_core(feature_map[b], boxes[b], gt_boxes[b], g0)
        rows = np.flatnonzero(np.any(Ghw != 0.0, axis=1))
        raw.append((fmhw, Ghw))
        rows_l.append(rows)
    nch = max(2, -(-max(r.shape[0] for r in rows_l) // KC))

    key = (dt_name, nch)
    if key not in _PROGRAMS:
        _PROGRAMS[key] = _build_program(dt_name, nch)
    nc = _PROGRAMS[key]

    in_maps = []
    for k in range(NCORES):
        FM, G = _pack_core(raw[k][0], raw[k][1], rows_l[k], nch, np_dt)
        in_maps.append({"fm": FM, "g": G})

    trace = bool(int(os.environ.get("ROI_TRACE", "0")))
    res = run_bass_kernel_spmd(nc, in_maps, list(range(NCORES)), trace=trace)
    LAST_RESULT = res

    out = np.zeros((B, N, C, P, P), dtype=np.float32)
    for k in range(NCORES):
        b = k // 4
        g0 = (k % 4) * GROUPS_PER_CORE
        out[b, g0:g0 + GROUPS_PER_CORE] = _unpack_core_out(res.results[k]["out"])
    return out


# revision 16
# speedup vs baseline: 1.0590x; 1.0162x over previous
"""ContextualRoIAlign Trainium2 kernel — fused group-kernel formulation.

Problem (hardcoded): B=2, C=256, H=W=56, N=64 boxes, M=8 gt boxes, P=7.
out[b,n,c,p,q] = roi_align(fm[b], box_n)[c,p,q]
                 + mean_m roi_align(fm[b], union(box_n, gt_m))[c,p,q]

roi_align separates per axis into interpolation matrices Ay, Ax
([7,dim], host-precomputed exactly like the reference), so each roi is
out_r = Ay_r @ fm @ Ax_r^T.  The whole 9-roi group sum (box + mean of
its 8 ctx unions, 1/M folded into Ax) collapses into ONE dense spatial
kernel per group:

    G_g[(h,w),(p,q)] = sum_j Ay_j[p,h] * Ax_j[q,w]          (host, ~44 MFLOP/core)
    out_g[c,(p,q)]   = sum_hw fm[c,(h,w)] * G_g[(h,w),(p,q)] (device)

The device then does a single [256 x 3136] @ [3136 x 784] matmul per
core at full 128x128 PE utilization: hw is chunked into 25 K-tiles of
128 accumulated in PSUM; fm chunk is the stationary operand (shared by
all 16 groups); G streams as the moving operand (784 = 16 groups x 49
output pixels, split in two 392-column halves to fit a PSUM bank).
~100 matmuls total instead of the ~2800 tiny per-roi matmuls of the
two-stage formulation.

Sharding: 8 cores; core k handles image k//4, box groups [16*(k%4), +16).

DMA: G ([3136, 784]) is the dominant stream; it is shipped in 5 large
super-chunks (5 K-tiles each) to stay near peak HBM bandwidth while
overlapping with PE compute. Default io dtype bf16 (G/fm magnitudes are
O(1); psum accumulates fp32); ROI_DTYPE=float32r keeps full precision
at 2x the DMA cost.
"""
import os
import numpy as np

P = 7
B, C, H, W, N, M = 2, 256, 56, 56, 64, 8
NCORES = 8
GROUPS_PER_CORE = 16
ROIS_PER_GROUP = 9          # 1 box + 8 ctx unions
R_CORE = GROUPS_PER_CORE * ROIS_PER_GROUP   # 144
HW = H * W                  # 3136
KC = 128                    # contraction tile (partition dim)
PQ = P * P                  # 49
COLS = GROUPS_PER_CORE * PQ             # 784 moving columns
COLH = COLS // 2                        # 392 (<=512 psum bank)


def _supers_for(nch):
    """DMA super-chunk schedule: geometrically growing so the PE starts
    early while steady-state transfers stay big (long contiguous
    descriptors, few completion latencies)."""
    lst = []
    s = 1
    rem = nch
    while rem > 0:
        s = min(s, rem)
        lst.append(s)
        rem -= s
        s = min(s * 2, 8)
    return tuple(lst)


# ---------------------------------------------------------------- host prep

def _axis_weights(start, length, dim):
    """Exact numpy port of the reference's _axis_weights (float32)."""
    start = start.astype(np.float32)
    length = length.astype(np.float32)
    R = start.shape[0]
    S = int(np.ceil(dim / P))
    bin_sz = length / np.float32(P)
    grid = np.ceil(length / np.float32(P)).astype(np.int32)
    g = grid.astype(np.float32)[:, None, None]
    s = np.arange(S, dtype=np.float32)
    ph = np.arange(P, dtype=np.float32)
    coord = (start[:, None, None] + ph[None, :, None] * bin_sz[:, None, None]
             + (s[None, None, :] + np.float32(0.5)) * bin_sz[:, None, None] / g)
    valid = (coord >= -1.0) & (coord <= dim)
    c = np.maximum(coord, np.float32(0.0))
    low = np.floor(c).astype(np.int32)
    hi_clamp = low >= dim - 1
    low = np.where(hi_clamp, dim - 1, low)
    high = np.where(hi_clamp, dim - 1, low + 1)
    cv = np.where(hi_clamp, low.astype(np.float32), c)
    l = cv - low.astype(np.float32)
    smask = (s[None, None, :] < g) & valid
    w = smask.astype(np.float32) / g
    w_low = ((np.float32(1.0) - l) * w).astype(np.float32)
    w_high = (l * w).astype(np.float32)
    A = np.zeros((R, P, dim), dtype=np.float32)
    r_idx = np.broadcast_to(np.arange(R)[:, None, None], low.shape)
    p_idx = np.broadcast_to(np.arange(P)[None, :, None], low.shape)
    np.add.at(A, (r_idx, p_idx, low), w_low)
    np.add.at(A, (r_idx, p_idx, high), w_high)
    return A


def _prep_core(fm_b, boxes_b, gt_b, g0):
    """Per-core raw arrays: fmhw [3136, 256], Ghw [3136, 784] (fp32)."""
    b = boxes_b.astype(np.float32)
    g = gt_b.astype(np.float32)
    x1 = np.minimum(b[:, None, 0], g[None, :, 0])
    y1 = np.minimum(b[:, None, 1], g[None, :, 1])
    x2 = np.maximum(b[:, None, 2], g[None, :, 2])
    y2 = np.maximum(b[:, None, 3], g[None, :, 3])
    ctx = np.stack([x1, y1, x2, y2], axis=-1)                 # [N,M,4]
    rois = np.concatenate([b[:, None, :], ctx], axis=1)       # [N,9,4]
    wts = np.full((N, ROIS_PER_GROUP), np.float32(1.0 / M), dtype=np.float32)
    wts[:, 0] = np.float32(1.0)

    rois = rois[g0:g0 + GROUPS_PER_CORE].reshape(R_CORE, 4)
    wts = wts[g0:g0 + GROUPS_PER_CORE].reshape(R_CORE)
    x1, y1, x2, y2 = rois[:, 0], rois[:, 1], rois[:, 2], rois[:, 3]
    roi_w = np.maximum(x2 - x1, np.float32(1.0))
    roi_h = np.maximum(y2 - y1, np.float32(1.0))
    Ay = _axis_weights(y1, roi_h, H)                          # [144,7,56]
    Ax = _axis_weights(x1, roi_w, W) * wts[:, None, None]     # [144,7,56]

    # G_g[p,h,q,w] = sum_j Ay[j,p,h] Ax[j,q,w]  (rank-9 per group)
    Ayg = Ay.reshape(GROUPS_PER_CORE, ROIS_PER_GROUP, P * H)
    Axg = Ax.reshape(GROUPS_PER_CORE, ROIS_PER_GROUP, P * W)
    G2 = np.matmul(Ayg.transpose(0, 2, 1), Axg)               # [16, 392, 392]
    G5 = G2.reshape(GROUPS_PER_CORE, P, H, P, W)
    # -> [(h,w), (g,p,q)]
    Ghw = np.ascontiguousarray(G5.transpose(2, 4, 0, 1, 3)).reshape(HW, COLS)
    fmhw = np.ascontiguousarray(fm_b.reshape(C, HW).T)        # [(h,w), c]
    return fmhw, Ghw


def _pack_core(fmhw, Ghw, rows, nch, np_dt):
    """Gather nonzero rows, pad to nch*128, tile to device layouts
    FM [128, nch, 256] and G [128, 2, nch, 392] (column-half major so
    each half streams as one contiguous DMA per ring)."""
    pad = nch * KC
    Gp = np.zeros((pad, COLS), dtype=np.float32)
    Gp[:rows.shape[0]] = Ghw[rows]
    Fp = np.zeros((pad, C), dtype=np.float32)
    Fp[:rows.shape[0]] = fmhw[rows]
    G = np.ascontiguousarray(
        Gp.reshape(nch, KC, 2, COLH).transpose(1, 2, 0, 3)).astype(np_dt)
    FM = np.ascontiguousarray(
        Fp.reshape(nch, KC, C).transpose(1, 0, 2)).astype(np_dt)
    return FM, G


def _unpack_core_out(OUT):
    """OUT [128, 2, 2, 392] -> [16, 256, 7, 7]."""
    a = np.asarray(OUT, dtype=np.float32)
    a = a.reshape(128, 2, 2, COLH).transpose(1, 0, 2, 3).reshape(C, COLS)
    a = a.reshape(C, GROUPS_PER_CORE, P, P).transpose(1, 0, 2, 3)
    return np.ascontiguousarray(a)


# ---------------------------------------------------------------- program

_PROGRAMS = {}


def _build_program(dt_name, nch):
    import concourse.bacc as bacc
    import concourse.tile as tile
    import concourse.mybir as mybir

    f32 = mybir.dt.float32
    dts = {"float32": mybir.dt.float32, "float32r": mybir.dt.float32r,
           "bfloat16": mybir.dt.bfloat16}
    io_dt = dts[dt_name]
    supers = _supers_for(nch)

    nc = bacc.Bacc("TRN2", target_bir_lowering=False, debug=False,
                   enable_asserts=False)
    fm_d = nc.dram_tensor("fm", [KC, nch, C], io_dt, kind="ExternalInput").ap()
    g_d = nc.dram_tensor("g", [KC, 2, nch, COLH], io_dt,
                         kind="ExternalInput").ap()
    out_d = nc.dram_tensor("out", [128, 2, 2, COLH], io_dt,
                           kind="ExternalOutput").ap()

    with tile.TileContext(nc) as tc:
        with tc.tile_pool(name="fmp", bufs=1) as fmp, \
             tc.tile_pool(name="gp", bufs=1) as gpool, \
             tc.tile_pool(name="outp", bufs=1) as opool, \
             tc.tile_pool(name="psp", bufs=1, space="PSUM") as psp:

            # stream inputs in super-chunks. The SDMA engines round-robin
            # between the two HWDGE rings (sync=SP, scalar=Act) at packet
            # granularity, so each G super-chunk is split by column half
            # across BOTH rings: the aggregate bandwidth always serves the
            # next-needed chunk and delivery stays in consumption order.
            fmt = []
            gt = []
            c0 = 0
            for i, s in enumerate(supers):
                qa = nc.sync if i % 2 == 0 else nc.scalar
                Fs = fmp.tile([KC, s, C], io_dt, name=f"fs{i}")
                qa.dma_start(Fs[:], fm_d[:, c0:c0 + s, :])
                Ga = gpool.tile([KC, s, COLH], io_dt, name=f"ga{i}")
                nc.scalar.dma_start(Ga[:], g_d[:, 0, c0:c0 + s, :])
                Gb = gpool.tile([KC, s, COLH], io_dt, name=f"gb{i}")
                nc.sync.dma_start(Gb[:], g_d[:, 1, c0:c0 + s, :])
                fmt.append(Fs)
                gt.append((Ga, Gb))
                c0 += s

            ps = [psp.tile([128, COLH], f32, name=f"ps{i}") for i in range(4)]
            OUTt = [opool.tile([128, COLH], io_dt, name=f"out{i}")
                    for i in range(4)]

            def drain(i):
                # psum -> bf16 sbuf -> hbm as soon as tile i's accumulation
                # closes; overlaps the remaining matmuls. All copies on DVE
                # (keeping the Act engine DMA-only avoids its act-table
                # load in the preamble).
                ch_, colh_ = divmod(i, 2)
                nc.vector.tensor_copy(out=OUTt[i][:], in_=ps[i][:])
                q = nc.sync if i % 2 == 0 else nc.scalar
                q.dma_start(out_d[:, ch_, colh_, :], OUTt[i][:])

            chunk = 0
            last_sup = len(supers) - 1
            for sup, s in enumerate(supers):
                if sup < last_sup:
                    for j in range(s):
                        for ch in range(2):
                            lhsT = fmt[sup][:, j, ch * 128:(ch + 1) * 128]
                            for colh in range(2):
                                nc.tensor.matmul(
                                    ps[ch * 2 + colh][:],
                                    lhsT,
                                    gt[sup][colh][:, j, :],
                                    start=(chunk == 0), stop=False)
                        chunk += 1
                else:
                    # last super-chunk: ch-major so ps[0]/ps[1] close (and
                    # start draining) while ps[2]/ps[3] still accumulate
                    for ch in range(2):
                        for j in range(s):
                            lhsT = fmt[sup][:, j, ch * 128:(ch + 1) * 128]
                            for colh in range(2):
                                nc.tensor.matmul(
                                    ps[ch * 2 + colh][:],
                                    lhsT,
                                    gt[sup][colh][:, j, :],
                                    start=False, stop=(j == s - 1))
                                if j == s - 1:
                                    drain(ch * 2 + colh)
                    chunk += s

    nc.compile()
    return nc


LAST_RESULT = None


def _ensure_axon_hooks_shim():
    """concourse's axon trace path imports antenv.axon_hooks, which this
    image's antenv package lacks; provide a minimal registry so a stray
    BASS_TRACE=1 in the environment cannot crash the kernel."""
    try:
        import antenv  # noqa: F401
        import antenv.axon_hooks  # noqa: F401
        return
    except ImportError:
        pass
    try:
        import sys
        import types
        import antenv
        mod = types.ModuleType("antenv.axon_hooks")
        mod._hook = None
        mod.get_axon_ntff_profile_hook = lambda: mod._hook

        def _set(h):
            mod._hook = h

        mod.set_axon_ntff_profile_hook = _set
        sys.modules["antenv.axon_hooks"] = mod
        antenv.axon_hooks = mod
    except Exception:
        pass


def kernel(feature_map, boxes, gt_boxes):
    global LAST_RESULT
    _ensure_axon_hooks_shim()
    feature_map = np.asarray(feature_map, dtype=np.float32)
    boxes = np.asarray(boxes, dtype=np.float32)
    gt_boxes = np.asarray(gt_boxes, dtype=np.float32)

    from concourse.bass_utils import run_bass_kernel_spmd

    dt_name = os.environ.get("ROI_DTYPE", "bfloat16")
    if dt_name == "bfloat16":
        import ml_dtypes
        np_dt = ml_dtypes.bfloat16
    else:
        np_dt = np.float32

    # host prep + row compaction: drop (h,w) rows where G is all-zero
    # (outside every roi's bilinear support); all cores share one program,
    # so the chunk count is the max over cores
    raw = []
    rows_l = []
    for k in range(NCORES):
        b = k // 4
        g0 = (k % 4) * GROUPS_PER_CORE
        fmhw, Ghw = _prep_core(feature_map[b], boxes[b], gt_boxes[b], g0)
        rows = np.flatnonzero(np.any(Ghw != 0.0, axis=1))
        raw.append((fmhw, Ghw))
        rows_l.append(rows)
    nch = max(2, -(-max(r.shape[0] for r in rows_l) // KC))

    key = (dt_name, nch)
    if key not in _PROGRAMS:
        _PROGRAMS[key] = _build_program(dt_name, nch)
    nc = _PROGRAMS[key]

    in_maps = []
    for k in range(NCORES):
        FM, G = _pack_core(raw[k][0], raw[k][1], rows_l[k], nch, np_dt)
        in_maps.append({"fm": FM, "g": G})

    trace = bool(int(os.environ.get("ROI_TRACE", "0")))
    res = run_bass_kernel_spmd(nc, in_maps, list(range(NCORES)), trace=trace)
    LAST_RESULT = res

    out = np.zeros((B, N, C, P, P), dtype=np.float32)
    for k in range(NCORES):
        b = k // 4
        g0 = (k % 4) * GROUPS_PER_CORE
        out[b, g0:g0 + GROUPS_PER_CORE] = _unpack_core_out(res.results[k]["out"])
    return out


# revision 19
# speedup vs baseline: 1.0872x; 1.0265x over previous
"""ContextualRoIAlign Trainium2 kernel — fused group-kernel formulation.

Problem (hardcoded): B=2, C=256, H=W=56, N=64 boxes, M=8 gt boxes, P=7.
out[b,n,c,p,q] = roi_align(fm[b], box_n)[c,p,q]
                 + mean_m roi_align(fm[b], union(box_n, gt_m))[c,p,q]

roi_align separates per axis into interpolation matrices Ay, Ax
([7,dim], host-precomputed exactly like the reference), so each roi is
out_r = Ay_r @ fm @ Ax_r^T.  The whole 9-roi group sum (box + mean of
its 8 ctx unions, 1/M folded into Ax) collapses into ONE dense spatial
kernel per group:

    G_g[(h,w),(p,q)] = sum_j Ay_j[p,h] * Ax_j[q,w]          (host, ~44 MFLOP/core)
    out_g[c,(p,q)]   = sum_hw fm[c,(h,w)] * G_g[(h,w),(p,q)] (device)

The device then does a single [256 x 3136] @ [3136 x 784] matmul per
core at full 128x128 PE utilization: hw is chunked into 25 K-tiles of
128 accumulated in PSUM; fm chunk is the stationary operand (shared by
all 16 groups); G streams as the moving operand (784 = 16 groups x 49
output pixels, split in two 392-column halves to fit a PSUM bank).
~100 matmuls total instead of the ~2800 tiny per-roi matmuls of the
two-stage formulation.

Sharding: 8 cores; core k handles image k//4, box groups [16*(k%4), +16).

DMA: G ([3136, 784]) is the dominant stream; it is shipped in 5 large
super-chunks (5 K-tiles each) to stay near peak HBM bandwidth while
overlapping with PE compute. Default io dtype bf16 (G/fm magnitudes are
O(1); psum accumulates fp32); ROI_DTYPE=float32r keeps full precision
at 2x the DMA cost.
"""
import os
import numpy as np

P = 7
B, C, H, W, N, M = 2, 256, 56, 56, 64, 8
NCORES = 8
GROUPS_PER_CORE = 16
ROIS_PER_GROUP = 9          # 1 box + 8 ctx unions
R_CORE = GROUPS_PER_CORE * ROIS_PER_GROUP   # 144
HW = H * W                  # 3136
KC = 128                    # contraction tile (partition dim)
PQ = P * P                  # 49
COLS = GROUPS_PER_CORE * PQ             # 784 moving columns
COLH = COLS // 2                        # 392 (<=512 psum bank)


def _supers_for(nch):
    """DMA super-chunk schedule: gradual growth matched to the HBM
    ramp-up so early chunks arrive as soon as possible, larger
    steady-state transfers once the stream is at line rate."""
    lst = []
    s = 1
    rem = nch
    while rem > 0:
        s = min(s, rem)
        lst.append(s)
        rem -= s
        s = min(s + 1, 5)
    return tuple(lst)


# ---------------------------------------------------------------- host prep

def _axis_weights(start, length, dim):
    """Exact numpy port of the reference's _axis_weights (float32)."""
    start = start.astype(np.float32)
    length = length.astype(np.float32)
    R = start.shape[0]
    S = int(np.ceil(dim / P))
    bin_sz = length / np.float32(P)
    grid = np.ceil(length / np.float32(P)).astype(np.int32)
    g = grid.astype(np.float32)[:, None, None]
    s = np.arange(S, dtype=np.float32)
    ph = np.arange(P, dtype=np.float32)
    coord = (start[:, None, None] + ph[None, :, None] * bin_sz[:, None, None]
             + (s[None, None, :] + np.float32(0.5)) * bin_sz[:, None, None] / g)
    valid = (coord >= -1.0) & (coord <= dim)
    c = np.maximum(coord, np.float32(0.0))
    low = np.floor(c).astype(np.int32)
    hi_clamp = low >= dim - 1
    low = np.where(hi_clamp, dim - 1, low)
    high = np.where(hi_clamp, dim - 1, low + 1)
    cv = np.where(hi_clamp, low.astype(np.float32), c)
    l = cv - low.astype(np.float32)
    smask = (s[None, None, :] < g) & valid
    w = smask.astype(np.float32) / g
    w_low = ((np.float32(1.0) - l) * w).astype(np.float32)
    w_high = (l * w).astype(np.float32)
    A = np.zeros((R, P, dim), dtype=np.float32)
    r_idx = np.broadcast_to(np.arange(R)[:, None, None], low.shape)
    p_idx = np.broadcast_to(np.arange(P)[None, :, None], low.shape)
    np.add.at(A, (r_idx, p_idx, low), w_low)
    np.add.at(A, (r_idx, p_idx, high), w_high)
    return A


def _prep_core(fm_b, boxes_b, gt_b, g0):
    """Per-core raw arrays: fmhw [3136, 256], Ghw [3136, 784] (fp32)."""
    b = boxes_b.astype(np.float32)
    g = gt_b.astype(np.float32)
    x1 = np.minimum(b[:, None, 0], g[None, :, 0])
    y1 = np.minimum(b[:, None, 1], g[None, :, 1])
    x2 = np.maximum(b[:, None, 2], g[None, :, 2])
    y2 = np.maximum(b[:, None, 3], g[None, :, 3])
    ctx = np.stack([x1, y1, x2, y2], axis=-1)                 # [N,M,4]
    rois = np.concatenate([b[:, None, :], ctx], axis=1)       # [N,9,4]
    wts = np.full((N, ROIS_PER_GROUP), np.float32(1.0 / M), dtype=np.float32)
    wts[:, 0] = np.float32(1.0)

    rois = rois[g0:g0 + GROUPS_PER_CORE].reshape(R_CORE, 4)
    wts = wts[g0:g0 + GROUPS_PER_CORE].reshape(R_CORE)
    x1, y1, x2, y2 = rois[:, 0], rois[:, 1], rois[:, 2], rois[:, 3]
    roi_w = np.maximum(x2 - x1, np.float32(1.0))
    roi_h = np.maximum(y2 - y1, np.float32(1.0))
    Ay = _axis_weights(y1, roi_h, H)                          # [144,7,56]
    Ax = _axis_weights(x1, roi_w, W) * wts[:, None, None]     # [144,7,56]

    # G_g[p,h,q,w] = sum_j Ay[j,p,h] Ax[j,q,w]  (rank-9 per group)
    Ayg = Ay.reshape(GROUPS_PER_CORE, ROIS_PER_GROUP, P * H)
    Axg = Ax.reshape(GROUPS_PER_CORE, ROIS_PER_GROUP, P * W)
    G2 = np.matmul(Ayg.transpose(0, 2, 1), Axg)               # [16, 392, 392]
    G5 = G2.reshape(GROUPS_PER_CORE, P, H, P, W)
    # -> [(h,w), (g,p,q)]
    Ghw = np.ascontiguousarray(G5.transpose(2, 4, 0, 1, 3)).reshape(HW, COLS)
    fmhw = np.ascontiguousarray(fm_b.reshape(C, HW).T)        # [(h,w), c]
    return fmhw, Ghw


def _pack_core(fmhw, Ghw, rows, nch, np_dt):
    """Gather nonzero rows, pad to nch*128, tile to device layouts
    FM [128, nch, 256] and G [128, 2, nch, 392] (column-half major so
    each half streams as one contiguous DMA per ring)."""
    pad = nch * KC
    Gp = np.zeros((pad, COLS), dtype=np.float32)
    Gp[:rows.shape[0]] = Ghw[rows]
    Fp = np.zeros((pad, C), dtype=np.float32)
    Fp[:rows.shape[0]] = fmhw[rows]
    G = np.ascontiguousarray(
        Gp.reshape(nch, KC, 2, COLH).transpose(1, 2, 0, 3)).astype(np_dt)
    FM = np.ascontiguousarray(
        Fp.reshape(nch, KC, C).transpose(1, 0, 2)).astype(np_dt)
    return FM, G


def _unpack_core_out(OUT):
    """OUT [128, 2, 2, 392] -> [16, 256, 7, 7]."""
    a = np.asarray(OUT, dtype=np.float32)
    a = a.reshape(128, 2, 2, COLH).transpose(1, 0, 2, 3).reshape(C, COLS)
    a = a.reshape(C, GROUPS_PER_CORE, P, P).transpose(1, 0, 2, 3)
    return np.ascontiguousarray(a)


# ---------------------------------------------------------------- program

_PROGRAMS = {}


def _build_program(dt_name, nch):
    import concourse.bacc as bacc
    import concourse.tile as tile
    import concourse.mybir as mybir

    f32 = mybir.dt.float32
    dts = {"float32": mybir.dt.float32, "float32r": mybir.dt.float32r,
           "bfloat16": mybir.dt.bfloat16}
    io_dt = dts[dt_name]
    supers = _supers_for(nch)

    nc = bacc.Bacc("TRN2", target_bir_lowering=False, debug=False,
                   enable_asserts=False)
    fm_d = nc.dram_tensor("fm", [KC, nch, C], io_dt, kind="ExternalInput").ap()
    g_d = nc.dram_tensor("g", [KC, 2, nch, COLH], io_dt,
                         kind="ExternalInput").ap()
    out_d = nc.dram_tensor("out", [128, 2, 2, COLH], io_dt,
                           kind="ExternalOutput").ap()

    with tile.TileContext(nc) as tc:
        with tc.tile_pool(name="fmp", bufs=1) as fmp, \
             tc.tile_pool(name="gp", bufs=1) as gpool, \
             tc.tile_pool(name="outp", bufs=1) as opool, \
             tc.tile_pool(name="psp", bufs=1, space="PSUM") as psp:

            # stream inputs in super-chunks. The SDMA engines round-robin
            # between the two HWDGE rings (sync=SP, scalar=Act) at packet
            # granularity, so each G super-chunk is split by column half
            # across BOTH rings: the aggregate bandwidth always serves the
            # next-needed chunk and delivery stays in consumption order.
            fmt = []
            gt = []
            c0 = 0
            for i, s in enumerate(supers):
                qa = nc.sync if i % 2 == 0 else nc.scalar
                Fs = fmp.tile([KC, s, C], io_dt, name=f"fs{i}")
                qa.dma_start(Fs[:], fm_d[:, c0:c0 + s, :])
                Ga = gpool.tile([KC, s, COLH], io_dt, name=f"ga{i}")
                nc.scalar.dma_start(Ga[:], g_d[:, 0, c0:c0 + s, :])
                Gb = gpool.tile([KC, s, COLH], io_dt, name=f"gb{i}")
                nc.sync.dma_start(Gb[:], g_d[:, 1, c0:c0 + s, :])
                fmt.append(Fs)
                gt.append((Ga, Gb))
                c0 += s

            ps = [psp.tile([128, COLH], f32, name=f"ps{i}") for i in range(4)]
            OUTt = [opool.tile([128, COLH], io_dt, name=f"out{i}")
                    for i in range(4)]

            # PE warmup: dummy matmuls fill the otherwise-idle DMA head so
            # the p-state governor reaches full clock before real work
            warm = fmp.tile([KC, COLH], io_dt, name="warm")
            nc.vector.memset(warm[:], 0.0)
            psw = psp.tile([16, COLH], f32, name="psw")
            for _ in range(20):
                nc.tensor.matmul(psw[:], warm[:, 0:16], warm[:],
                                 start=True, stop=True)

            def drain(i):
                # psum -> bf16 sbuf -> hbm as soon as tile i's accumulation
                # closes; overlaps the remaining matmuls. All copies on DVE
                # (keeping the Act engine DMA-only avoids its act-table
                # load in the preamble).
                ch_, colh_ = divmod(i, 2)
                nc.vector.tensor_copy(out=OUTt[i][:], in_=ps[i][:])
                q = nc.sync if i % 2 == 0 else nc.scalar
                q.dma_start(out_d[:, ch_, colh_, :], OUTt[i][:])

            chunk = 0
            last_sup = len(supers) - 1
            for sup, s in enumerate(supers):
                if sup < last_sup:
                    for j in range(s):
                        for ch in range(2):
                            lhsT = fmt[sup][:, j, ch * 128:(ch + 1) * 128]
                            for colh in range(2):
                                nc.tensor.matmul(
                                    ps[ch * 2 + colh][:],
                                    lhsT,
                                    gt[sup][colh][:, j, :],
                                    start=(chunk == 0), stop=False)
                        chunk += 1
                else:
                    # last super-chunk: ch-major so ps[0]/ps[1] close (and
                    # start draining) while ps[2]/ps[3] still accumulate
                    for ch in range(2):
                        for j in range(s):
                            lhsT = fmt[sup][:, j, ch * 128:(ch + 1) * 128]
                            for colh in range(2):
                                nc.tensor.matmul(
                                    ps[ch * 2 + colh][:],
                                    lhsT,
                                    gt[sup][colh][:, j, :],
                                    start=False, stop=(j == s - 1))
                                if j == s - 1:
                                    drain(ch * 2 + colh)
                    chunk += s

    nc.compile()
    return nc


LAST_RESULT = None


def _ensure_axon_hooks_shim():
    """concourse's axon trace path imports antenv.axon_hooks, which this
    image's antenv package lacks; provide a minimal registry so a stray
    BASS_TRACE=1 in the environment cannot crash the kernel."""
    try:
        import antenv  # noqa: F401
        import antenv.axon_hooks  # noqa: F401
        return
    except ImportError:
        pass
    try:
        import sys
        import types
        import antenv
        mod = types.ModuleType("antenv.axon_hooks")
        mod._hook = None
        mod.get_axon_ntff_profile_hook = lambda: mod._hook

        def _set(h):
            mod._hook = h

        mod.set_axon_ntff_profile_hook = _set
        sys.modules["antenv.axon_hooks"] = mod
        antenv.axon_hooks = mod
    except Exception:
        pass


def kernel(feature_map, boxes, gt_boxes):
    global LAST_RESULT
    _ensure_axon_hooks_shim()
    feature_map = np.asarray(feature_map, dtype=np.float32)
    boxes = np.asarray(boxes, dtype=np.float32)
    gt_boxes = np.asarray(gt_boxes, dtype=np.float32)

    from concourse.bass_utils import run_bass_kernel_spmd

    dt_name = os.environ.get("ROI_DTYPE", "bfloat16")
    if dt_name == "bfloat16":
        import ml_dtypes
        np_dt = ml_dtypes.bfloat16
    else:
        np_dt = np.float32

    # host prep + row compaction: drop (h,w) rows where G is all-zero
    # (outside every roi's bilinear support); all cores share one program,
    # so the chunk count is the max over cores
    raw = []
    rows_l = []
    for k in range(NCORES):
        b = k // 4
        g0 = (k % 4) * GROUPS_PER_CORE
        fmhw, Ghw = _prep_core(feature_map[b], boxes[b], gt_boxes[b], g0)
        rows = np.flatnonzero(np.any(Ghw != 0.0, axis=1))
        raw.append((fmhw, Ghw))
        rows_l.append(rows)
    nch = max(2, -(-max(r.shape[0] for r in rows_l) // KC))

    key = (dt_name, nch)
    if key not in _PROGRAMS:
        _PROGRAMS[key] = _build_program(dt_name, nch)
    nc = _PROGRAMS[key]

    in_maps = []
    for k in range(NCORES):
        FM, G = _pack_core(raw[k][0], raw[k][1], rows_l[k], nch, np_dt)
        in_maps.append({"fm": FM, "g": G})

    trace = bool(int(os.environ.get("ROI_TRACE", "0")))
    res = run_bass_kernel_spmd(nc, in_maps, list(range(NCORES)), trace=trace)
    LAST_RESULT = res

    out = np.zeros((B, N, C, P, P), dtype=np.float32)
    for k in range(NCORES):
        b = k // 4
        g0 = (k % 4) * GROUPS_PER_CORE
        out[b, g0:g0 + GROUPS_PER_CORE] = _unpack_core_out(res.results[k]["out"])
    return out
